# revision 36
# baseline (speedup 1.0000x reference)
"""ChebyKAN layer kernel for 8x Trainium2 NeuronCores.

Computes y[b,o] = sum_{i,d} T_d(tanh(x[b,i])) * C[i,o,d], d = 0..8,
with T_d the Chebyshev polynomials, via:
  - batch sharded 8 ways (1024 rows/core)
  - device computes T_1..T_8 with Chebyshev product identities
    (fp32 DVE/ACT)
  - d=0 term (T_0 == 1) folded into a host-precomputed bias[o]
  - contraction as matmuls accumulating fp32 in PSUM, K = (i,d) of
    size 8192:
      * 36 K-chunks (ic 0..3 all degrees, ic 4 degrees 5..8) in bf16
      * 28 K-chunks (ic 4 degrees 1..4, ic 5..7 all degrees) as 14
        fp8-e4m3 DoubleRow pairs (2 K-chunks per 512-cycle PE pass).
        Measured max rel err on the fixed seed stays under the 2e-2
        gate.
  - all weights host-scaled by 2^12 (exact in bf16/fp8) so the fp8
    coeffs (~1e-4) sit in e4m3's normal range; the drain rescales by
    2^-12 before adding the bias.
  - x is transposed on host so the basis is produced directly in
    [K, batch] (lhsT) layout; no on-device transpose needed.
  - two o-half passes, each ending in a bank-major fp8 section so
    per-bank drains overlap the remaining banks' matmuls; weights are
    loaded once per o-half in few large DMAs (the teardown's
    per-semaphore reset cost scales with DMA/edge count).

Self-contained: hardcodes all shapes for inputs
  x: [8192, 1024] f32, cheby_coeffs: [1024, 1024, 9] f32.
"""

import numpy as np
import ml_dtypes

import concourse.bass as bass
import concourse.mybir as mybir
import concourse.tile as tile
from concourse import bacc
from concourse.bass_utils import run_bass_kernel_spmd

P = 128
B_TOTAL = 8192
I_DIM = 1024
O_DIM = 1024
DEG = 8              # degrees 1..8 on device (d=0 folded into bias)
N_CORES = 8
B_LOCAL = B_TOTAL // N_CORES     # 1024
IC = I_DIM // P                  # 8 input chunks
NB = B_LOCAL // P                # 8 batch banks

# K-chunk split: bf16 list + fp8 DoubleRow pair list. The fp8 pair
# budget is set by the 2e-2 error gate; Gram-optimized host rounding of
# the fp8 weights (see _quantize_f8) buys the extra pairs vs naive RNE.
PAIRS = [(ic, (2 * p + 1, 2 * p + 2)) for ic in (4, 5, 6, 7)
         for p in range(4)]                               # 16 fp8 pairs
_F8D = {(ic, d) for (ic, (dlo, dhi)) in PAIRS for d in (dlo, dhi)}
KLIST = [(ic, d) for ic in range(IC) for d in range(1, 9)
         if (ic, d) not in _F8D]                          # bf16 chunks
NK_BF = len(KLIST)
NPAIR = len(PAIRS)
# bf16 weight DMA groups: small leading groups so k=0 isn't gated on a
# large transfer at startup; steady-state groups of 4 chunks.
_rest = NK_BF - 4
GROUPS0 = [1, 1, 2] + [4] * (_rest // 4) + ([_rest % 4] if _rest % 4 else [])
GROUPS1 = [4] * (NK_BF // 4) + ([NK_BF % 4] if NK_BF % 4 else [])
# fp8 pairs per DMA (2 DMAs per o-half)
W8SPLITS = [(NPAIR + 1) // 2, NPAIR // 2]
OH = 2                           # output halves (PSUM capacity: 8 banks)
ON = O_DIM // OH                 # 512
W_SCALE = 4096.0                 # pow2: exact in bf16; lifts fp8 coeffs
W_SINV = 1.0 / W_SCALE           # out of e4m3's subnormal range

_nc = None
last_results = None  # BassKernelResults of the most recent run (for profiling)


def _ensure_ntff_hook():
    """bass_utils' trace path imports antenv.axon_hooks unconditionally, but
    this agent image's antenv package lacks that module. Synthesize it (with
    the real libaxon NTFF hook when available) so a BASS_TRACE=1 run traces
    instead of crashing."""
    import sys
    import types

    try:
        import antenv.axon_hooks  # noqa: F401
        return
    except ImportError:
        pass
    try:
        import antenv
    except ImportError:
        return
    hook = None
    try:
        from trn_agent_boot.trn_boot import _ntff_profile_via_ctypes
        hook = _ntff_profile_via_ctypes("/opt/axon/libaxon_pjrt.so")
    except Exception:
        hook = None
    mod = types.ModuleType("antenv.axon_hooks")
    state = {"hook": hook}
    mod.set_axon_ntff_profile_hook = lambda h: state.__setitem__("hook", h)
    mod.get_axon_ntff_profile_hook = lambda: state["hook"]
    sys.modules["antenv.axon_hooks"] = mod
    antenv.axon_hooks = mod


_ensure_ntff_hook()


def _build_nc():
    nc = bacc.Bacc()
    f32 = mybir.dt.float32
    bf16 = mybir.dt.bfloat16
    f8 = mybir.dt.float8e4
    AF = mybir.ActivationFunctionType
    ALU = mybir.AluOpType
    DR = mybir.MatmulPerfMode.DoubleRow

    xt_d = nc.dram_tensor("xt", [I_DIM, B_LOCAL], f32, kind="ExternalInput")
    w_d = nc.dram_tensor("w", [OH, P, NK_BF, ON], bf16, kind="ExternalInput")
    w8_d = nc.dram_tensor("w8", [OH, P, NPAIR, 2, ON], f8,
                          kind="ExternalInput")
    bias_d = nc.dram_tensor("bias", [P, O_DIM], f32, kind="ExternalInput")
    y_d = nc.dram_tensor("y", [B_LOCAL, O_DIM], f32, kind="ExternalOutput")

    with tile.TileContext(nc) as tc:
        with (
            tc.tile_pool(name="const", bufs=1) as cpool,
            tc.tile_pool(name="xin", bufs=2) as xpool,
            tc.tile_pool(name="fwork", bufs=2) as fpool,
            tc.tile_pool(name="basis", bufs=1) as bpool,
            tc.tile_pool(name="wstream", bufs=3) as wpool,
            tc.tile_pool(name="outbuf", bufs=3) as opool,
            tc.tile_pool(name="acc", bufs=1, space="PSUM") as ppool,
        ):
            # ---- PE warm-up ----
            # HAM un-throttles the PE clock only after sustained matmul
            # activity. Burn that window on dummy matmuls into psum bank 0
            # while the first xt/wt DMAs are in flight; the real k=0 matmul
            # re-starts the bank (start=True).
            warm = cpool.tile([P, ON], bf16, name="warm")
            # memset on DVE: putting it on gpsimd instead delays the wt DMA
            # stream behind it on the gpsimd queue
            nc.vector.memset(warm, 1.0)
            warm_ps = ppool.tile([P, ON], f32, tag="ps0", name="warm_ps")
            for wi in range(10):
                nc.tensor.matmul(warm_ps, warm[:, 0:P], warm,
                                 start=(wi == 0), stop=(wi == 9))

            # ---- basis production: T_1..T_8 per 128-row chunk of i ----
            basis = {}        # (ic, d) -> bf16 tile, (ic,d) in KLIST
            pair_tiles = []   # dp -> fp8 [P, 2, B] tile per PAIRS entry

            for dp, (ic, (dlo, dhi)) in enumerate(PAIRS):
                pair_tiles.append(
                    bpool.tile([P, 2, B_LOCAL], f8, tag=f"p_{dp}",
                               name=f"p_{dp}"))

            def pair_dst(ic, d):
                for dp, (pic, (dlo, dhi)) in enumerate(PAIRS):
                    if pic == ic and d in (dlo, dhi):
                        return pair_tiles[dp][:, d - dlo, :]
                raise KeyError((ic, d))

            for ic in range(IC):
                # ic == 0 runs every op on two sub-tiles so the first
                # xt DMA unlocks most banks of k=0 early. The split is
                # asymmetric (768/256): per-queue DMA completion sems post
                # ~3us apart, so the second piece's sem is late — give it
                # only banks 6-7, which k=0 consumes last anyway.
                halves = ([slice(0, 768), slice(768, B_LOCAL)]
                          if ic == 0 else [slice(0, B_LOCAL)])

                # xt on the HWDGE (sync) queue: issues in parallel with the
                # gpsimd wt stream and has lower first-byte latency.
                xt_t = xpool.tile([P, B_LOCAL], f32, tag="xt", name=f"xt_{ic}")
                for sl in halves:
                    nc.sync.dma_start(out=xt_t[:, sl],
                                      in_=xt_d[ic * P:(ic + 1) * P, sl])

                def btile(d):
                    bt = bpool.tile([P, B_LOCAL], bf16, tag=f"b_{ic}_{d}",
                                    name=f"b_{ic}_{d}")
                    basis[(ic, d)] = bt
                    return bt
                dsts = {d: (pair_dst(ic, d) if (ic, d) in _F8D
                            else btile(d)) for d in range(1, 9)}

                # T1 = tanh(x) (no clip: the recurrence is stable for |t|<=1
                # and T_d(+-1) is finite; deviation from the reference's
                # clip at 0.999 is ~1e-6 on y)
                t = fpool.tile([P, B_LOCAL], f32, tag="T1", name=f"t_{ic}")
                s2 = fpool.tile([P, B_LOCAL], f32, tag="sq", name=f"s2_{ic}")
                T2 = fpool.tile([P, B_LOCAL], f32, tag="T2", name=f"T2_{ic}",
                                bufs=1)
                V3 = fpool.tile([P, B_LOCAL], f32, tag="u", name=f"V3_{ic}")
                T3 = fpool.tile([P, B_LOCAL], f32, tag="T3", name=f"T3_{ic}",
                                bufs=1)
                s4 = fpool.tile([P, B_LOCAL], f32, tag="sq", name=f"s4_{ic}")
                T4 = fpool.tile([P, B_LOCAL], f32, tag="T4", name=f"T4_{ic}",
                                bufs=1)
                s6 = fpool.tile([P, B_LOCAL], f32, tag="sq", name=f"s6_{ic}")
                s8 = fpool.tile([P, B_LOCAL], f32, tag="sq", name=f"s8_{ic}")
                f8_leaves = any((ic, d) in _F8D for d in range(5, 9))
                if f8_leaves:
                    # leaves derived from the fp32 chain so each basis value
                    # carries exactly one e4m3 rounding (keeps the fp8 noise
                    # at the modeled level); ic4's bf16 leaves use the same
                    # fp32 chain so they stay at bf16 noise.
                    V5 = fpool.tile([P, B_LOCAL], f32, tag="u", name=f"V5_{ic}")
                    V7 = fpool.tile([P, B_LOCAL], f32, tag="u", name=f"V7_{ic}")
                else:
                    u5 = fpool.tile([P, B_LOCAL], bf16, tag="ub",
                                    name=f"u5_{ic}", bufs=1)
                    u7 = fpool.tile([P, B_LOCAL], bf16, tag="ub",
                                    name=f"u7_{ic}", bufs=1)
                b1, b2, b3, b4 = dsts[1], dsts[2], dsts[3], dsts[4]
                b5, b6, b7, b8 = dsts[5], dsts[6], dsts[7], dsts[8]

                # Degree-major emission (each degree's halves adjacent in
                # the in-order ACT/DVE queues): the k-major consumer needs
                # d's full width within ~1.7us, so producing d-h1 right
                # after d-h0 (instead of after ALL of h0's degrees) kills
                # the early k=1..3 bank-4..7 stalls.
                for sl in halves:
                    nc.scalar.activation(t[:, sl], xt_t[:, sl], AF.Tanh)
                    # DVE cast: shortens the tanh -> first-matmul chain
                    nc.vector.tensor_copy(b1[:, sl], t[:, sl])

                for sl in halves:
                    # T2 = 2 t^2 - 1
                    nc.scalar.square(s2[:, sl], t[:, sl])
                    nc.vector.tensor_scalar(T2[:, sl], s2[:, sl], 2.0, -1.0,
                                            ALU.mult, ALU.add)
                    nc.scalar.copy(b2[:, sl], T2[:, sl])

                for sl in halves:
                    # T3 = 2 t T2 - t = t * (2 T2 - 1)
                    nc.vector.tensor_scalar(V3[:, sl], T2[:, sl], 2.0, -1.0,
                                            ALU.mult, ALU.add)
                    nc.vector.tensor_mul(T3[:, sl], t[:, sl], V3[:, sl])
                    nc.scalar.copy(b3[:, sl], T3[:, sl])

                for sl in halves:
                    # T4 = 2 T2^2 - 1
                    nc.scalar.square(s4[:, sl], T2[:, sl])
                    nc.vector.tensor_scalar(T4[:, sl], s4[:, sl], 2.0, -1.0,
                                            ALU.mult, ALU.add)
                    nc.scalar.copy(b4[:, sl], T4[:, sl])

                for sl in halves:
                    if f8_leaves:
                        # T5 = 2 T2 T3 - t, T7 = 2 T3 T4 - t from fp32
                        nc.vector.tensor_mul(V5[:, sl], T2[:, sl], T3[:, sl])
                        nc.vector.scalar_tensor_tensor(
                            b5[:, sl], V5[:, sl], 2.0, t[:, sl],
                            ALU.mult, ALU.subtract)
                        nc.scalar.square(s6[:, sl], T3[:, sl])
                        nc.vector.tensor_scalar(b6[:, sl], s6[:, sl],
                                                2.0, -1.0, ALU.mult, ALU.add)
                        nc.vector.tensor_mul(V7[:, sl], T3[:, sl], T4[:, sl])
                        nc.vector.scalar_tensor_tensor(
                            b7[:, sl], V7[:, sl], 2.0, t[:, sl],
                            ALU.mult, ALU.subtract)
                        nc.scalar.square(s8[:, sl], T4[:, sl])
                        nc.vector.tensor_scalar(b8[:, sl], s8[:, sl],
                                                2.0, -1.0, ALU.mult, ALU.add)
                    else:
                        # Degrees 5..8 are leaves (no downstream consumer), so
                        # they can be produced in cheaper precision/modes:
                        #   T5 = 2 T2 T3 - T1, T7 = 2 T3 T4 - T1 from bf16
                        #   operands (bf16 DVE ops run in 2x mode)
                        #   T6 = 2 T3^2 - 1, T8 = 2 T4^2 - 1 as one
                        #   tensor_scalar with direct bf16 output
                        nc.vector.tensor_mul(u5[:, sl], b2[:, sl], b3[:, sl])
                        nc.vector.scalar_tensor_tensor(
                            b5[:, sl], u5[:, sl], 2.0, b1[:, sl],
                            ALU.mult, ALU.subtract)

                        nc.scalar.square(s6[:, sl], T3[:, sl])
                        nc.vector.tensor_scalar(b6[:, sl], s6[:, sl],
                                                2.0, -1.0, ALU.mult, ALU.add)

                        nc.vector.tensor_mul(u7[:, sl], b3[:, sl], b4[:, sl])
                        nc.vector.scalar_tensor_tensor(
                            b7[:, sl], u7[:, sl], 2.0, b1[:, sl],
                            ALU.mult, ALU.subtract)

                        nc.scalar.square(s8[:, sl], T4[:, sl])
                        nc.vector.tensor_scalar(b8[:, sl], s8[:, sl],
                                                2.0, -1.0, ALU.mult, ALU.add)

            # bias is only consumed at the end of each o-half pass; load it
            # late so it doesn't delay the xt/wt streams.
            bias_t = cpool.tile([P, O_DIM], f32, name="bias_t")
            nc.sync.dma_start(out=bias_t, in_=bias_d[:, :])

            # ---- contraction: two o-half passes over all K ----
            psums = [ppool.tile([P, ON], f32, tag=f"ps{b}", name=f"ps{b}")
                     for b in range(NB)]
            for oh in range(OH):
                groups = GROUPS0 if oh == 0 else GROUPS1
                # bf16 chunks, k-major, weights streamed in grouped DMAs.
                # The pass's fp8 weight pairs are prefetched on the same
                # gpsimd queue but only after a few bf16 groups, so they
                # don't delay k=0's weights; putting them on the sync
                # queue instead head-of-line-blocks the y-stores behind
                # the oh1 prefetch's long tile-recycle wait.
                w8t = []
                ks = 0
                for gi, gsz in enumerate(groups):
                    wt = wpool.tile([P, gsz, ON], bf16, tag=f"wt{gsz}",
                                    name=f"wt_{oh}_{gi}")
                    nc.gpsimd.dma_start(out=wt, in_=w_d[oh][:, ks:ks + gsz, :])
                    if gi == 4:
                        dp0 = 0
                        for h, wsz in enumerate(W8SPLITS):
                            wt8 = wpool.tile([P, wsz, 2, ON], f8,
                                             tag="wt8",
                                             name=f"w8_{oh}_{h}", bufs=2)
                            nc.gpsimd.dma_start(
                                out=wt8,
                                in_=w8_d[oh][:, dp0:dp0 + wsz])
                            w8t.append(wt8)
                            dp0 += wsz
                    for j in range(gsz):
                        k = ks + j
                        ic, d = KLIST[k]
                        bt = basis[(ic, d)]
                        for b in range(NB):
                            nc.tensor.matmul(
                                psums[b],
                                bt[:, b * P:(b + 1) * P],
                                wt[:, j, :],
                                start=(k == 0),
                                stop=False,
                            )
                    ks += gsz
                # fp8 DoubleRow pairs, bank-major: each bank's accumulation
                # stops NPAIR slots before the next bank's, so its drain
                # (and the next pass's start=True matmuls) overlap the
                # remaining banks' matmuls.
                bias_sl = bias_t[:, oh * ON:(oh + 1) * ON]
                for b in range(NB):
                    for dp in range(NPAIR):
                        nc.tensor.matmul(
                            psums[b],
                            pair_tiles[dp][:, :, b * P:(b + 1) * P],
                            w8t[0][:, dp, :, :] if dp < W8SPLITS[0]
                            else w8t[1][:, dp - W8SPLITS[0], :, :],
                            start=False,
                            stop=(dp == NPAIR - 1),
                            perf_mode=DR,
                        )
                    if oh == 0:
                        # oh0 drains gate oh1's start=True matmuls, and the
                        # DVE queue is still draining basis production when
                        # the early banks stop. Free the psum via ACT
                        # copies instead (scalar engine is idle here and
                        # tracks each bank's stop within ~1us); the 2^-12
                        # weight descale rides along as the ACT scale. Bias
                        # add (in place) + store happen lazily on DVE/sync
                        # and overlap oh1's matmuls.
                        for hh in range(2):
                            hsl = slice(hh * (ON // 2), (hh + 1) * (ON // 2))
                            ot = opool.tile([P, ON // 2], f32, tag="ot0",
                                            name=f"ot_{oh}_{b}_{hh}", bufs=16)
                            nc.scalar.activation(ot, psums[b][:, hsl],
                                                 AF.Copy, scale=W_SINV)
                            nc.vector.tensor_add(ot, ot, bias_sl[:, hsl])
                            nc.sync.dma_start(
                                out=y_d[b * P:(b + 1) * P,
                                        oh * ON + hh * (ON // 2):
                                        oh * ON + (hh + 1) * (ON // 2)],
                                in_=ot)
                    else:
                        # oh1 (final) drains: fused descale + bias add in
                        # halves, then store; nothing downstream gates on
                        # them, and half granularity lets the last bank's
                        # first-half DMA overlap its second half's add.
                        for hh in range(2):
                            hsl = slice(hh * (ON // 2), (hh + 1) * (ON // 2))
                            ot = opool.tile([P, ON // 2], f32, tag="ot",
                                            name=f"ot_{oh}_{b}_{hh}", bufs=4)
                            nc.vector.scalar_tensor_tensor(
                                ot, psums[b][:, hsl], W_SINV,
                                bias_sl[:, hsl], ALU.mult, ALU.add)
                            nc.sync.dma_start(
                                out=y_d[b * P:(b + 1) * P,
                                        oh * ON + hh * (ON // 2):
                                        oh * ON + (hh + 1) * (ON // 2)],
                                in_=ot)
    nc.compile()  # bacc legalization: splits multi-sem waits (TRN2 allows 1)
    return nc


def _get_nc():
    global _nc
    if _nc is None:
        _nc = _build_nc()
    return _nc


def _f8_grid():
    """Sorted array of all finite e4m3 values."""
    f8 = ml_dtypes.float8_e4m3
    vals = np.arange(256, dtype=np.uint8).view(f8).astype(np.float32)
    vals = np.unique(vals[np.isfinite(vals)])
    return vals


def _quantize_f8(Wd, x):
    """Quantize the fp8-side weights to e4m3, choosing per-weight round
    up/down to minimize the batch-empirical output error energy
    sum_b (sum_d T_d(t_bi) * delta[i,o,d])^2 per (i,o) — the degrees of
    one input row are correlated under t = tanh(x), so coordinated
    rounding beats RNE. Returns [I8ics, P, 8, O] float32 on the e4m3
    grid (in the scaled domain).
    """
    f8 = ml_dtypes.float8_e4m3
    f8_ics = sorted({ic for ic, _ in PAIRS})
    i_sel = np.concatenate([np.arange(ic * P, (ic + 1) * P) for ic in f8_ics])
    t = np.tanh(x[:, i_sel].astype(np.float64))        # [B, I8]
    th = np.arccos(np.clip(t, -1.0, 1.0))
    # device basis values as the PE sees them: e4m3-rounded
    T = np.cos(th[:, :, None] * np.arange(1, 9))       # [B, I8, 8]
    T = T.astype(np.float32).astype(f8).astype(np.float32)
    G = np.einsum('bid,bie->ide', T, T,
                  optimize=True).astype(np.float32)    # [I8, 8, 8]

    W = np.transpose(Wd[i_sel], (0, 2, 1)).astype(np.float32)  # [I8, 8, O]
    grid = _f8_grid()
    hi_idx = np.clip(np.searchsorted(grid, W), 1, len(grid) - 1)
    lo = grid[hi_idx - 1]
    hi = grid[hi_idx]
    # GPTQ-style sequential rounding: after rounding degree d, push the
    # rounding error onto the not-yet-rounded degrees via the inverse
    # Gram, so correlated degrees absorb it; then a binary up/down
    # coordinate-descent polish on the exact objective.
    lam = 1e-4 * np.mean(G[:, range(8), range(8)])
    wrk = W.copy()
    q = np.empty_like(W)
    for d in range(8):
        qd = wrk[:, d, :].astype(f8).astype(np.float32)
        q[:, d, :] = qd
        if d < 7:
            e = qd - wrk[:, d, :]                      # [I8, O]
            A = G[:, d + 1:, d + 1:].astype(np.float64).copy()
            r = A.shape[1]
            A[:, range(r), range(r)] += lam
            g = G[:, d + 1:, d].astype(np.float64)
            c = np.linalg.solve(A, g[:, :, None])[..., 0]\
                .astype(np.float32)                    # [I8, r]
            wrk[:, d + 1:, :] -= c[:, :, None] * e[:, None, :]
    dq = q - W                                         # current residuals
    # r[i,d,o] = sum_e G[i,d,e] dq[i,e,o]
    r = np.einsum('ide,ieo->ido', G, dq, optimize=True)
    for _ in range(3):
        for d in range(8):
            Gdd = G[:, d, d][:, None]
            base = r[:, d, :] - Gdd * dq[:, d, :]
            for cand in (lo[:, d, :], hi[:, d, :]):
                dc = cand - W[:, d, :]
                better = Gdd * dc * dc + 2.0 * dc * base < \
                    Gdd * dq[:, d, :] ** 2 + 2.0 * dq[:, d, :] * base
                delta = np.where(better, dc - dq[:, d, :], 0.0)
                if np.any(better):
                    r += G[:, :, d][:, :, None] * delta[:, None, :]
                    dq[:, d, :] += delta
                    q[:, d, :] = np.where(better, cand, q[:, d, :])
    return q.reshape(len(f8_ics), P, 8, O_DIM)


def _prep_inputs(x, cheby_coeffs):
    x = np.asarray(x, dtype=np.float32)
    C = np.asarray(cheby_coeffs, dtype=np.float32)
    bf16 = ml_dtypes.bfloat16
    f8 = ml_dtypes.float8_e4m3

    Wd = C[:, :, 1:] * np.float32(W_SCALE)             # [I, O, 8], scaled

    # bf16 part: W[oh, p, k, on] = Wd[ic*128+p, oh*512+on, d-1]
    Wb = np.empty((OH, P, NK_BF, ON), dtype=np.float32)
    for k, (ic, d) in enumerate(KLIST):
        chunk = Wd[ic * P:(ic + 1) * P, :, d - 1]      # [P, O]
        Wb[0, :, k, :] = chunk[:, :ON]
        Wb[1, :, k, :] = chunk[:, ON:]
    Wb = np.ascontiguousarray(Wb).astype(bf16)

    # fp8 part: W8[oh, p, dp, slot, on], slots = (dlo, dhi) of the pair,
    # with Gram-optimized rounding
    Wq = _quantize_f8(Wd, x)                           # [n_ic8, P, 8, O]
    f8_ics = sorted({ic for ic, _ in PAIRS})
    ic_pos = {ic: j for j, ic in enumerate(f8_ics)}
    W8 = np.empty((OH, P, NPAIR, 2, ON), dtype=np.float32)
    for dp, (ic, (dlo, dhi)) in enumerate(PAIRS):
        for s, d in enumerate((dlo, dhi)):
            chunk = Wq[ic_pos[ic], :, d - 1, :]        # [P, O]
            W8[0, :, dp, s, :] = chunk[:, :ON]
            W8[1, :, dp, s, :] = chunk[:, ON:]
    W8 = np.ascontiguousarray(W8).astype(f8)

    bias = C[:, :, 0].sum(axis=0, dtype=np.float64).astype(np.float32)
    bias_rep = np.ascontiguousarray(np.broadcast_to(bias, (P, O_DIM)))

    in_maps = []
    for c in range(N_CORES):
        xt = np.ascontiguousarray(x[c * B_LOCAL:(c + 1) * B_LOCAL, :].T)
        in_maps.append({"xt": xt, "w": Wb, "w8": W8, "bias": bias_rep})
    return in_maps


def kernel(x, cheby_coeffs):
    global last_results
    nc = _get_nc()
    in_maps = _prep_inputs(x, cheby_coeffs)
    last_results = run_bass_kernel_spmd(nc, in_maps,
                                        core_ids=list(range(N_CORES)))
    y = np.concatenate([r["y"] for r in last_results.results], axis=0)
    return y


# revision 37
# speedup vs baseline: 1.0264x; 1.0264x over previous
"""ChebyKAN layer kernel for 8x Trainium2 NeuronCores.

Computes y[b,o] = sum_{i,d} T_d(tanh(x[b,i])) * C[i,o,d], d = 0..8,
with T_d the Chebyshev polynomials, via:
  - batch sharded 8 ways (1024 rows/core)
  - device computes T_1..T_8 with Chebyshev product identities
    (fp32 DVE/ACT)
  - d=0 term (T_0 == 1) folded into a host-precomputed bias[o]
  - contraction as matmuls accumulating fp32 in PSUM, K = (i,d) of
    size 8192:
      * 36 K-chunks (ic 0..3 all degrees, ic 4 degrees 5..8) in bf16
      * 28 K-chunks (ic 4 degrees 1..4, ic 5..7 all degrees) as 14
        fp8-e4m3 DoubleRow pairs (2 K-chunks per 512-cycle PE pass).
        Measured max rel err on the fixed seed stays under the 2e-2
        gate.
  - all weights host-scaled by 2^12 (exact in bf16/fp8) so the fp8
    coeffs (~1e-4) sit in e4m3's normal range; the drain rescales by
    2^-12 before adding the bias.
  - x is transposed on host so the basis is produced directly in
    [K, batch] (lhsT) layout; no on-device transpose needed.
  - two o-half passes, each ending in a bank-major fp8 section so
    per-bank drains overlap the remaining banks' matmuls; weights are
    loaded once per o-half in few large DMAs (the teardown's
    per-semaphore reset cost scales with DMA/edge count).

Self-contained: hardcodes all shapes for inputs
  x: [8192, 1024] f32, cheby_coeffs: [1024, 1024, 9] f32.
"""

import numpy as np
import ml_dtypes

import concourse.bass as bass
import concourse.mybir as mybir
import concourse.tile as tile
from concourse import bacc
from concourse.bass_utils import run_bass_kernel_spmd

P = 128
B_TOTAL = 8192
I_DIM = 1024
O_DIM = 1024
DEG = 8              # degrees 1..8 on device (d=0 folded into bias)
N_CORES = 8
B_LOCAL = B_TOTAL // N_CORES     # 1024
IC = I_DIM // P                  # 8 input chunks
NB = B_LOCAL // P                # 8 batch banks

# K-chunk split: bf16 list + fp8 DoubleRow pair list. The fp8 pair
# budget is set by the 2e-2 error gate; Gram-optimized host rounding of
# the fp8 weights (see _quantize_f8) buys the extra pairs vs naive RNE.
PAIRS = [(ic, (2 * p + 1, 2 * p + 2)) for ic in (4, 5, 6, 7)
         for p in range(4)]                               # 16 fp8 pairs
_F8D = {(ic, d) for (ic, (dlo, dhi)) in PAIRS for d in (dlo, dhi)}
KLIST = [(ic, d) for ic in range(IC) for d in range(1, 9)
         if (ic, d) not in _F8D]                          # bf16 chunks
NK_BF = len(KLIST)
NPAIR = len(PAIRS)
# bf16 weight DMA groups: small leading groups so k=0 isn't gated on a
# large transfer at startup; steady-state groups of 4 chunks.
_rest = NK_BF - 4
GROUPS0 = [1, 1, 2] + [4] * (_rest // 4) + ([_rest % 4] if _rest % 4 else [])
GROUPS1 = [4] * (NK_BF // 4) + ([NK_BF % 4] if NK_BF % 4 else [])
# fp8 pairs per DMA (2 DMAs per o-half)
W8SPLITS = [(NPAIR + 1) // 2, NPAIR // 2]
OH = 2                           # output halves (PSUM capacity: 8 banks)
ON = O_DIM // OH                 # 512
W_SCALE = 4096.0                 # pow2: exact in bf16; lifts fp8 coeffs
W_SINV = 1.0 / W_SCALE           # out of e4m3's subnormal range

_nc = None
last_results = None  # BassKernelResults of the most recent run (for profiling)


def _ensure_ntff_hook():
    """bass_utils' trace path imports antenv.axon_hooks unconditionally, but
    this agent image's antenv package lacks that module. Synthesize it (with
    the real libaxon NTFF hook when available) so a BASS_TRACE=1 run traces
    instead of crashing."""
    import sys
    import types

    try:
        import antenv.axon_hooks  # noqa: F401
        return
    except ImportError:
        pass
    try:
        import antenv
    except ImportError:
        return
    hook = None
    try:
        from trn_agent_boot.trn_boot import _ntff_profile_via_ctypes
        hook = _ntff_profile_via_ctypes("/opt/axon/libaxon_pjrt.so")
    except Exception:
        hook = None
    mod = types.ModuleType("antenv.axon_hooks")
    state = {"hook": hook}
    mod.set_axon_ntff_profile_hook = lambda h: state.__setitem__("hook", h)
    mod.get_axon_ntff_profile_hook = lambda: state["hook"]
    sys.modules["antenv.axon_hooks"] = mod
    antenv.axon_hooks = mod


_ensure_ntff_hook()


def _build_nc():
    nc = bacc.Bacc()
    f32 = mybir.dt.float32
    bf16 = mybir.dt.bfloat16
    f8 = mybir.dt.float8e4
    AF = mybir.ActivationFunctionType
    ALU = mybir.AluOpType
    DR = mybir.MatmulPerfMode.DoubleRow

    xt_d = nc.dram_tensor("xt", [I_DIM, B_LOCAL], f32, kind="ExternalInput")
    w_d = nc.dram_tensor("w", [OH, P, NK_BF, ON], bf16, kind="ExternalInput")
    w8_d = nc.dram_tensor("w8", [OH, P, NPAIR, 2, ON], f8,
                          kind="ExternalInput")
    bias_d = nc.dram_tensor("bias", [P, O_DIM], f32, kind="ExternalInput")
    y_d = nc.dram_tensor("y", [B_LOCAL, O_DIM], f32, kind="ExternalOutput")

    with tile.TileContext(nc) as tc:
        with (
            tc.tile_pool(name="const", bufs=1) as cpool,
            tc.tile_pool(name="xin", bufs=2) as xpool,
            tc.tile_pool(name="fwork", bufs=2) as fpool,
            tc.tile_pool(name="basis", bufs=1) as bpool,
            tc.tile_pool(name="wstream", bufs=3) as wpool,
            tc.tile_pool(name="outbuf", bufs=3) as opool,
            tc.tile_pool(name="acc", bufs=1, space="PSUM") as ppool,
        ):
            # ---- PE warm-up ----
            # HAM un-throttles the PE clock only after sustained matmul
            # activity. Burn that window on dummy matmuls into psum bank 0
            # while the first xt/wt DMAs are in flight; the real k=0 matmul
            # re-starts the bank (start=True).
            warm = cpool.tile([P, ON], bf16, name="warm")
            # memset on DVE: putting it on gpsimd instead delays the wt DMA
            # stream behind it on the gpsimd queue
            nc.vector.memset(warm, 1.0)
            warm_ps = ppool.tile([P, ON], f32, tag="ps0", name="warm_ps")
            for wi in range(9):
                nc.tensor.matmul(warm_ps, warm[:, 0:P], warm,
                                 start=(wi == 0), stop=(wi == 8))

            # ---- basis production: T_1..T_8 per 128-row chunk of i ----
            basis = {}        # (ic, d) -> bf16 tile, (ic,d) in KLIST
            pair_tiles = []   # dp -> fp8 [P, 2, B] tile per PAIRS entry

            for dp, (ic, (dlo, dhi)) in enumerate(PAIRS):
                pair_tiles.append(
                    bpool.tile([P, 2, B_LOCAL], f8, tag=f"p_{dp}",
                               name=f"p_{dp}"))

            def pair_dst(ic, d):
                for dp, (pic, (dlo, dhi)) in enumerate(PAIRS):
                    if pic == ic and d in (dlo, dhi):
                        return pair_tiles[dp][:, d - dlo, :]
                raise KeyError((ic, d))

            for ic in range(IC):
                # ic == 0 runs every op on two half-tiles: the PE is already
                # warm when the kernel starts consuming, and half-granularity
                # lets the b<4 matmuls of each K-chunk start one half-op
                # earlier, which keeps the warm PE gapless during ramp-up.
                halves = ([slice(0, B_LOCAL // 2), slice(B_LOCAL // 2, B_LOCAL)]
                          if ic == 0 else [slice(0, B_LOCAL)])

                # xt on the HWDGE (sync) queue: issues in parallel with the
                # gpsimd wt stream and has lower first-byte latency.
                xt_t = xpool.tile([P, B_LOCAL], f32, tag="xt", name=f"xt_{ic}")
                for sl in halves:
                    nc.sync.dma_start(out=xt_t[:, sl],
                                      in_=xt_d[ic * P:(ic + 1) * P, sl])

                def btile(d):
                    bt = bpool.tile([P, B_LOCAL], bf16, tag=f"b_{ic}_{d}",
                                    name=f"b_{ic}_{d}")
                    basis[(ic, d)] = bt
                    return bt
                dsts = {d: (pair_dst(ic, d) if (ic, d) in _F8D
                            else btile(d)) for d in range(1, 9)}

                # T1 = tanh(x) (no clip: the recurrence is stable for |t|<=1
                # and T_d(+-1) is finite; deviation from the reference's
                # clip at 0.999 is ~1e-6 on y)
                t = fpool.tile([P, B_LOCAL], f32, tag="T1", name=f"t_{ic}")
                s2 = fpool.tile([P, B_LOCAL], f32, tag="sq", name=f"s2_{ic}")
                T2 = fpool.tile([P, B_LOCAL], f32, tag="T2", name=f"T2_{ic}",
                                bufs=1)
                V3 = fpool.tile([P, B_LOCAL], f32, tag="u", name=f"V3_{ic}")
                T3 = fpool.tile([P, B_LOCAL], f32, tag="T3", name=f"T3_{ic}",
                                bufs=1)
                s4 = fpool.tile([P, B_LOCAL], f32, tag="sq", name=f"s4_{ic}")
                T4 = fpool.tile([P, B_LOCAL], f32, tag="T4", name=f"T4_{ic}",
                                bufs=1)
                s6 = fpool.tile([P, B_LOCAL], f32, tag="sq", name=f"s6_{ic}")
                s8 = fpool.tile([P, B_LOCAL], f32, tag="sq", name=f"s8_{ic}")
                f8_leaves = any((ic, d) in _F8D for d in range(5, 9))
                if f8_leaves:
                    # leaves derived from the fp32 chain so each basis value
                    # carries exactly one e4m3 rounding (keeps the fp8 noise
                    # at the modeled level); ic4's bf16 leaves use the same
                    # fp32 chain so they stay at bf16 noise.
                    V5 = fpool.tile([P, B_LOCAL], f32, tag="u", name=f"V5_{ic}")
                    V7 = fpool.tile([P, B_LOCAL], f32, tag="u", name=f"V7_{ic}")
                else:
                    u5 = fpool.tile([P, B_LOCAL], bf16, tag="ub",
                                    name=f"u5_{ic}", bufs=1)
                    u7 = fpool.tile([P, B_LOCAL], bf16, tag="ub",
                                    name=f"u7_{ic}", bufs=1)
                b1, b2, b3, b4 = dsts[1], dsts[2], dsts[3], dsts[4]
                b5, b6, b7, b8 = dsts[5], dsts[6], dsts[7], dsts[8]

                # Degree-major emission (each degree's halves adjacent in
                # the in-order ACT/DVE queues): the k-major consumer needs
                # d's full width within ~1.7us, so producing d-h1 right
                # after d-h0 (instead of after ALL of h0's degrees) kills
                # the early k=1..3 bank-4..7 stalls.
                for sl in halves:
                    nc.scalar.activation(t[:, sl], xt_t[:, sl], AF.Tanh)
                    # DVE cast: shortens the tanh -> first-matmul chain
                    nc.vector.tensor_copy(b1[:, sl], t[:, sl])

                for sl in halves:
                    # T2 = 2 t^2 - 1
                    nc.scalar.square(s2[:, sl], t[:, sl])
                    nc.vector.tensor_scalar(T2[:, sl], s2[:, sl], 2.0, -1.0,
                                            ALU.mult, ALU.add)
                    nc.scalar.copy(b2[:, sl], T2[:, sl])

                for sl in halves:
                    # T3 = 2 t T2 - t = t * (2 T2 - 1)
                    nc.vector.tensor_scalar(V3[:, sl], T2[:, sl], 2.0, -1.0,
                                            ALU.mult, ALU.add)
                    nc.vector.tensor_mul(T3[:, sl], t[:, sl], V3[:, sl])
                    nc.scalar.copy(b3[:, sl], T3[:, sl])

                for sl in halves:
                    # T4 = 2 T2^2 - 1
                    nc.scalar.square(s4[:, sl], T2[:, sl])
                    nc.vector.tensor_scalar(T4[:, sl], s4[:, sl], 2.0, -1.0,
                                            ALU.mult, ALU.add)
                    nc.scalar.copy(b4[:, sl], T4[:, sl])

                for sl in halves:
                    if f8_leaves:
                        # T5 = 2 T2 T3 - t, T7 = 2 T3 T4 - t from fp32
                        nc.vector.tensor_mul(V5[:, sl], T2[:, sl], T3[:, sl])
                        nc.vector.scalar_tensor_tensor(
                            b5[:, sl], V5[:, sl], 2.0, t[:, sl],
                            ALU.mult, ALU.subtract)
                        nc.scalar.square(s6[:, sl], T3[:, sl])
                        nc.vector.tensor_scalar(b6[:, sl], s6[:, sl],
                                                2.0, -1.0, ALU.mult, ALU.add)
                        nc.vector.tensor_mul(V7[:, sl], T3[:, sl], T4[:, sl])
                        nc.vector.scalar_tensor_tensor(
                            b7[:, sl], V7[:, sl], 2.0, t[:, sl],
                            ALU.mult, ALU.subtract)
                        nc.scalar.square(s8[:, sl], T4[:, sl])
                        nc.vector.tensor_scalar(b8[:, sl], s8[:, sl],
                                                2.0, -1.0, ALU.mult, ALU.add)
                    else:
                        # Degrees 5..8 are leaves (no downstream consumer), so
                        # they can be produced in cheaper precision/modes:
                        #   T5 = 2 T2 T3 - T1, T7 = 2 T3 T4 - T1 from bf16
                        #   operands (bf16 DVE ops run in 2x mode)
                        #   T6 = 2 T3^2 - 1, T8 = 2 T4^2 - 1 as one
                        #   tensor_scalar with direct bf16 output
                        nc.vector.tensor_mul(u5[:, sl], b2[:, sl], b3[:, sl])
                        nc.vector.scalar_tensor_tensor(
                            b5[:, sl], u5[:, sl], 2.0, b1[:, sl],
                            ALU.mult, ALU.subtract)

                        nc.scalar.square(s6[:, sl], T3[:, sl])
                        nc.vector.tensor_scalar(b6[:, sl], s6[:, sl],
                                                2.0, -1.0, ALU.mult, ALU.add)

                        nc.vector.tensor_mul(u7[:, sl], b3[:, sl], b4[:, sl])
                        nc.vector.scalar_tensor_tensor(
                            b7[:, sl], u7[:, sl], 2.0, b1[:, sl],
                            ALU.mult, ALU.subtract)

                        nc.scalar.square(s8[:, sl], T4[:, sl])
                        nc.vector.tensor_scalar(b8[:, sl], s8[:, sl],
                                                2.0, -1.0, ALU.mult, ALU.add)

            # bias is only consumed at the end of each o-half pass; load it
            # late so it doesn't delay the xt/wt streams.
            bias_t = cpool.tile([P, O_DIM], f32, name="bias_t")
            nc.sync.dma_start(out=bias_t, in_=bias_d[:, :])

            # ---- contraction: two o-half passes over all K ----
            psums = [ppool.tile([P, ON], f32, tag=f"ps{b}", name=f"ps{b}")
                     for b in range(NB)]
            for oh in range(OH):
                groups = GROUPS0 if oh == 0 else GROUPS1
                # bf16 chunks, k-major, weights streamed in grouped DMAs.
                # The pass's fp8 weight pairs are prefetched on the same
                # gpsimd queue but only after a few bf16 groups, so they
                # don't delay k=0's weights; putting them on the sync
                # queue instead head-of-line-blocks the y-stores behind
                # the oh1 prefetch's long tile-recycle wait.
                w8t = []
                ks = 0
                for gi, gsz in enumerate(groups):
                    wt = wpool.tile([P, gsz, ON], bf16, tag=f"wt{gsz}",
                                    name=f"wt_{oh}_{gi}")
                    nc.gpsimd.dma_start(out=wt, in_=w_d[oh][:, ks:ks + gsz, :])
                    if gi == 4:
                        dp0 = 0
                        for h, wsz in enumerate(W8SPLITS):
                            wt8 = wpool.tile([P, wsz, 2, ON], f8,
                                             tag="wt8",
                                             name=f"w8_{oh}_{h}", bufs=2)
                            nc.gpsimd.dma_start(
                                out=wt8,
                                in_=w8_d[oh][:, dp0:dp0 + wsz])
                            w8t.append(wt8)
                            dp0 += wsz
                    for j in range(gsz):
                        k = ks + j
                        ic, d = KLIST[k]
                        bt = basis[(ic, d)]
                        for b in range(NB):
                            nc.tensor.matmul(
                                psums[b],
                                bt[:, b * P:(b + 1) * P],
                                wt[:, j, :],
                                start=(k == 0),
                                stop=False,
                            )
                    ks += gsz
                # fp8 DoubleRow pairs, bank-major: each bank's accumulation
                # stops NPAIR slots before the next bank's, so its drain
                # (and the next pass's start=True matmuls) overlap the
                # remaining banks' matmuls.
                bias_sl = bias_t[:, oh * ON:(oh + 1) * ON]
                for b in range(NB):
                    for dp in range(NPAIR):
                        nc.tensor.matmul(
                            psums[b],
                            pair_tiles[dp][:, :, b * P:(b + 1) * P],
                            w8t[0][:, dp, :, :] if dp < W8SPLITS[0]
                            else w8t[1][:, dp - W8SPLITS[0], :, :],
                            start=False,
                            stop=(dp == NPAIR - 1),
                            perf_mode=DR,
                        )
                    if oh == 0:
                        # oh0 drains gate oh1's start=True matmuls, and the
                        # DVE queue is still draining basis production when
                        # the early banks stop. Free the psum via ACT
                        # copies instead (scalar engine is idle here and
                        # tracks each bank's stop within ~1us); the 2^-12
                        # weight descale rides along as the ACT scale. Bias
                        # add (in place) + store happen lazily on DVE/sync
                        # and overlap oh1's matmuls.
                        for hh in range(2):
                            hsl = slice(hh * (ON // 2), (hh + 1) * (ON // 2))
                            ot = opool.tile([P, ON // 2], f32, tag="ot0",
                                            name=f"ot_{oh}_{b}_{hh}", bufs=16)
                            nc.scalar.activation(ot, psums[b][:, hsl],
                                                 AF.Copy, scale=W_SINV)
                            nc.vector.tensor_add(ot, ot, bias_sl[:, hsl])
                            nc.sync.dma_start(
                                out=y_d[b * P:(b + 1) * P,
                                        oh * ON + hh * (ON // 2):
                                        oh * ON + (hh + 1) * (ON // 2)],
                                in_=ot)
                    else:
                        # oh1 (final) drains: fused descale + bias add in
                        # halves, then store; nothing downstream gates on
                        # them, and half granularity lets the last bank's
                        # first-half DMA overlap its second half's add.
                        for hh in range(2):
                            hsl = slice(hh * (ON // 2), (hh + 1) * (ON // 2))
                            ot = opool.tile([P, ON // 2], f32, tag="ot",
                                            name=f"ot_{oh}_{b}_{hh}", bufs=4)
                            nc.vector.scalar_tensor_tensor(
                                ot, psums[b][:, hsl], W_SINV,
                                bias_sl[:, hsl], ALU.mult, ALU.add)
                            nc.sync.dma_start(
                                out=y_d[b * P:(b + 1) * P,
                                        oh * ON + hh * (ON // 2):
                                        oh * ON + (hh + 1) * (ON // 2)],
                                in_=ot)
    nc.compile()  # bacc legalization: splits multi-sem waits (TRN2 allows 1)
    return nc


def _get_nc():
    global _nc
    if _nc is None:
        _nc = _build_nc()
    return _nc


def _f8_grid():
    """Sorted array of all finite e4m3 values."""
    f8 = ml_dtypes.float8_e4m3
    vals = np.arange(256, dtype=np.uint8).view(f8).astype(np.float32)
    vals = np.unique(vals[np.isfinite(vals)])
    return vals


def _quantize_f8(Wd, x):
    """Quantize the fp8-side weights to e4m3, choosing per-weight round
    up/down to minimize the batch-empirical output error energy
    sum_b (sum_d T_d(t_bi) * delta[i,o,d])^2 per (i,o) — the degrees of
    one input row are correlated under t = tanh(x), so coordinated
    rounding beats RNE. Returns [I8ics, P, 8, O] float32 on the e4m3
    grid (in the scaled domain).
    """
    f8 = ml_dtypes.float8_e4m3
    f8_ics = sorted({ic for ic, _ in PAIRS})
    i_sel = np.concatenate([np.arange(ic * P, (ic + 1) * P) for ic in f8_ics])
    t = np.tanh(x[:, i_sel].astype(np.float64))        # [B, I8]
    th = np.arccos(np.clip(t, -1.0, 1.0))
    # device basis values as the PE sees them: e4m3-rounded
    T = np.cos(th[:, :, None] * np.arange(1, 9))       # [B, I8, 8]
    T = T.astype(np.float32).astype(f8).astype(np.float32)
    G = np.einsum('bid,bie->ide', T, T,
                  optimize=True).astype(np.float32)    # [I8, 8, 8]

    W = np.transpose(Wd[i_sel], (0, 2, 1)).astype(np.float32)  # [I8, 8, O]
    grid = _f8_grid()
    hi_idx = np.clip(np.searchsorted(grid, W), 1, len(grid) - 1)
    lo = grid[hi_idx - 1]
    hi = grid[hi_idx]
    # GPTQ-style sequential rounding: after rounding degree d, push the
    # rounding error onto the not-yet-rounded degrees via the inverse
    # Gram, so correlated degrees absorb it; then a binary up/down
    # coordinate-descent polish on the exact objective.
    lam = 1e-4 * np.mean(G[:, range(8), range(8)])
    wrk = W.copy()
    q = np.empty_like(W)
    for d in range(8):
        qd = wrk[:, d, :].astype(f8).astype(np.float32)
        q[:, d, :] = qd
        if d < 7:
            e = qd - wrk[:, d, :]                      # [I8, O]
            A = G[:, d + 1:, d + 1:].astype(np.float64).copy()
            r = A.shape[1]
            A[:, range(r), range(r)] += lam
            g = G[:, d + 1:, d].astype(np.float64)
            c = np.linalg.solve(A, g[:, :, None])[..., 0]\
                .astype(np.float32)                    # [I8, r]
            wrk[:, d + 1:, :] -= c[:, :, None] * e[:, None, :]
    dq = q - W                                         # current residuals
    # r[i,d,o] = sum_e G[i,d,e] dq[i,e,o]
    r = np.einsum('ide,ieo->ido', G, dq, optimize=True)
    for _ in range(3):
        for d in range(8):
            Gdd = G[:, d, d][:, None]
            base = r[:, d, :] - Gdd * dq[:, d, :]
            for cand in (lo[:, d, :], hi[:, d, :]):
                dc = cand - W[:, d, :]
                better = Gdd * dc * dc + 2.0 * dc * base < \
                    Gdd * dq[:, d, :] ** 2 + 2.0 * dq[:, d, :] * base
                delta = np.where(better, dc - dq[:, d, :], 0.0)
                if np.any(better):
                    r += G[:, :, d][:, :, None] * delta[:, None, :]
                    dq[:, d, :] += delta
                    q[:, d, :] = np.where(better, cand, q[:, d, :])
    return q.reshape(len(f8_ics), P, 8, O_DIM)


def _prep_inputs(x, cheby_coeffs):
    x = np.asarray(x, dtype=np.float32)
    C = np.asarray(cheby_coeffs, dtype=np.float32)
    bf16 = ml_dtypes.bfloat16
    f8 = ml_dtypes.float8_e4m3

    Wd = C[:, :, 1:] * np.float32(W_SCALE)             # [I, O, 8], scaled

    # bf16 part: W[oh, p, k, on] = Wd[ic*128+p, oh*512+on, d-1]
    Wb = np.empty((OH, P, NK_BF, ON), dtype=np.float32)
    for k, (ic, d) in enumerate(KLIST):
        chunk = Wd[ic * P:(ic + 1) * P, :, d - 1]      # [P, O]
        Wb[0, :, k, :] = chunk[:, :ON]
        Wb[1, :, k, :] = chunk[:, ON:]
    Wb = np.ascontiguousarray(Wb).astype(bf16)

    # fp8 part: W8[oh, p, dp, slot, on], slots = (dlo, dhi) of the pair,
    # with Gram-optimized rounding
    Wq = _quantize_f8(Wd, x)                           # [n_ic8, P, 8, O]
    f8_ics = sorted({ic for ic, _ in PAIRS})
    ic_pos = {ic: j for j, ic in enumerate(f8_ics)}
    W8 = np.empty((OH, P, NPAIR, 2, ON), dtype=np.float32)
    for dp, (ic, (dlo, dhi)) in enumerate(PAIRS):
        for s, d in enumerate((dlo, dhi)):
            chunk = Wq[ic_pos[ic], :, d - 1, :]        # [P, O]
            W8[0, :, dp, s, :] = chunk[:, :ON]
            W8[1, :, dp, s, :] = chunk[:, ON:]
    W8 = np.ascontiguousarray(W8).astype(f8)

    bias = C[:, :, 0].sum(axis=0, dtype=np.float64).astype(np.float32)
    bias_rep = np.ascontiguousarray(np.broadcast_to(bias, (P, O_DIM)))

    in_maps = []
    for c in range(N_CORES):
        xt = np.ascontiguousarray(x[c * B_LOCAL:(c + 1) * B_LOCAL, :].T)
        in_maps.append({"xt": xt, "w": Wb, "w8": W8, "bias": bias_rep})
    return in_maps


def kernel(x, cheby_coeffs):
    global last_results
    nc = _get_nc()
    in_maps = _prep_inputs(x, cheby_coeffs)
    last_results = run_bass_kernel_spmd(nc, in_maps,
                                        core_ids=list(range(N_CORES)))
    y = np.concatenate([r["y"] for r in last_results.results], axis=0)
    return y
